# revision 8
# baseline (speedup 1.0000x reference)
"""Trainium2 Bass kernel for nn_DeconvBlock (dynamic-weight transposed conv).

Computes, per sample b:
    w_b   = weight + sum_j feature[b,j] * (t_j * m_j)            (weight synthesis)
    out_b = conv_transpose2d(x_b, w_b, stride=2, pad=1, K=4)     (grouped over batch)
    out   = prelu(out_b + bias, a)

Strategy (data-parallel over batch, 8 cores x 2 samples):
  - conv_transpose(stride 2, K=4, P=1) decomposes into 4 output phases
    (py,px) in {0,1}^2; each phase output pixel is a sum of 4 "taps"
    (ky,kx), each tap a 1x1 conv (matmul over CIN=256) of a +-1 shifted x.
  - Weights are synthesized on-device on VectorE with fused
    scalar_tensor_tensor (acc = TM_j * f_bj + acc).
  - Matmuls run as float32r (FP22 multiply, fp32 accumulate) which is
    full-rate on TRN2 for moving dim >= 256.
  - Epilogue: ScalarE adds bias (Identity activation w/ per-partition
    bias), VectorE computes prelu(t) = max(t, a*t) in one fused op while
    interleaving the 4 phases into contiguous output rows for clean DMA.
"""

import numpy as np

import concourse.bass as bass
import concourse.mybir as mybir
from concourse import bacc
from concourse import bass_utils
from concourse.tile import TileContext

B, CIN, COUT, H, W, K, S = 16, 256, 128, 64, 64, 4, 2
NCORES = 8
BPC = B // NCORES  # samples per core
P = 128
NCH = CIN // P     # ic chunks of 128
HP = H + 2         # padded x height/width (zero border of 1)
NROW = 8           # output-phase rows per block
NYB = H // NROW    # row blocks per sample

# phase py -> ((ky, sy), ...): contribution x[y'+sy] * w[ky]
_TAPS = {0: ((1, 0), (3, -1)), 1: ((2, 0), (0, 1))}

_COMPILED = None


def _build():
    f32 = mybir.dt.float32
    f32r = mybir.dt.float32r
    Alu = mybir.AluOpType
    Act = mybir.ActivationFunctionType

    nc = bacc.Bacc(
        "TRN2", target_bir_lowering=False, debug=False, num_devices=NCORES
    )
    x_d = nc.dram_tensor(
        "x_sh", (BPC, NCH, P, HP, HP), f32, kind="ExternalInput"
    ).ap()
    w5_d = nc.dram_tensor("w5", (P, 5, NCH, K, K, COUT), f32, kind="ExternalInput").ap()
    feat_d = nc.dram_tensor("featb", (P, BPC, 4), f32, kind="ExternalInput").ap()
    bias_d = nc.dram_tensor("biasb", (P, 1), f32, kind="ExternalInput").ap()
    a_d = nc.dram_tensor("ab", (P, 1), f32, kind="ExternalInput").ap()
    out_d = nc.dram_tensor(
        "out_sh", (BPC, COUT, H * S, W * S), f32, kind="ExternalOutput"
    ).ap()

    with TileContext(nc) as tc:
        with (
            tc.tile_pool(name="const", bufs=1) as const_pool,
            tc.tile_pool(name="tmj_pool", bufs=3) as tm_pool,
            tc.tile_pool(name="wsyn_pool", bufs=1) as wsyn_pool,
            tc.tile_pool(name="x_pool", bufs=1) as x_pool,
            tc.tile_pool(name="t_pool", bufs=4) as t_pool,
            tc.tile_pool(name="row_pool", bufs=3) as row_pool,
            tc.tile_pool(name="psum", bufs=6, space="PSUM") as psum_pool,
        ):
            feat_t = const_pool.tile([P, BPC, 4], f32)
            nc.sync.dma_start(feat_t[:], feat_d[:])
            bias_t = const_pool.tile([P, 1], f32)
            nc.sync.dma_start(bias_t[:], bias_d[:])
            a_t = const_pool.tile([P, 1], f32)
            nc.sync.dma_start(a_t[:], a_d[:])

            # ---- per-sample weight synthesis on VectorE ----
            # w_syn[s][p, c, ky, kx, oc] = base + sum_j f[s,j] * TM_j
            wsyn = []
            for s in range(BPC):
                w_s = wsyn_pool.tile(
                    [P, NCH, K, K, COUT], f32r, name=f"wsyn{s}", tag=f"wsyn{s}"
                )
                wsyn.append(w_s)
                for c in range(NCH):
                    nc.sync.dma_start(w_s[:, c], w5_d[:, 4, c].bitcast(f32r))
            for c in range(NCH):
                for j in range(4):
                    tmj = tm_pool.tile([P, K, K, COUT], f32r, name="tmj", tag="tmj")
                    nc.sync.dma_start(tmj[:], w5_d[:, j, c].bitcast(f32r))
                    for s in range(BPC):
                        nc.vector.scalar_tensor_tensor(
                            wsyn[s][:, c],
                            tmj[:],
                            feat_t[:, s, j : j + 1],
                            wsyn[s][:, c],
                            op0=Alu.mult,
                            op1=Alu.add,
                        )

            # ---- load x with zero border padding ----
            xt = []
            for s in range(BPC):
                x_s = x_pool.tile(
                    [P, NCH, HP, HP], f32r, name=f"xpad{s}", tag=f"xpad{s}"
                )
                xt.append(x_s)
                for c in range(NCH):
                    nc.sync.dma_start(x_s[:, c], x_d[s, c].bitcast(f32r))

            # ---- main conv loop ----
            for s in range(BPC):
                for yb in range(NYB):
                    # row_t free layout (y', py, x', px) == out rows
                    # [2*NROW, 2*W] for oy in [2*NROW*yb, 2*NROW*(yb+1))
                    row_t = row_pool.tile(
                        [P, NROW, 2, W, 2], f32, name="row_t", tag="row_t"
                    )
                    for py in (0, 1):
                        for px in (0, 1):
                            ps = psum_pool.tile(
                                [P, NROW, W], f32, name="ps", tag="ps"
                            )
                            k = 0
                            for c in range(NCH):
                                for ky, sy in _TAPS[py]:
                                    for kx, sx in _TAPS[px]:
                                        lhsT = wsyn[s][:, c, ky, kx, :]
                                        y0 = 1 + sy + NROW * yb
                                        x0 = 1 + sx
                                        rhs = xt[s][
                                            :, c, y0 : y0 + NROW, x0 : x0 + W
                                        ]
                                        nc.tensor.matmul(
                                            ps[:],
                                            lhsT,
                                            rhs,
                                            start=(k == 0),
                                            stop=(k == 7),
                                        )
                                        k += 1
                            tt = t_pool.tile([P, NROW, W], f32, name="tt", tag="tt")
                            nc.scalar.activation(
                                tt[:], ps[:], Act.Identity, bias=bias_t[:], scale=1.0
                            )
                            # prelu(t) = max(t, a*t), interleaved into row_t
                            nc.vector.scalar_tensor_tensor(
                                row_t[:, :, py, :, px],
                                tt[:],
                                a_t[:],
                                tt[:],
                                op0=Alu.mult,
                                op1=Alu.max,
                            )
                    oy0 = 2 * NROW * yb
                    nc.sync.dma_start(
                        out_d[s, :, oy0 : oy0 + 2 * NROW, :], row_t[:]
                    )

    nc.compile()
    return nc


def _get_compiled():
    global _COMPILED
    if _COMPILED is None:
        _COMPILED = _build()
    return _COMPILED


def _prep_in_maps(inputs):
    x = np.asarray(inputs["x"], dtype=np.float32)
    xp = np.zeros((B, NCH, P, HP, HP), dtype=np.float32)
    xp[:, :, :, 1 : HP - 1, 1 : HP - 1] = x.reshape(B, NCH, P, H, W)
    feat = np.asarray(inputs["feature"], dtype=np.float32)
    w = np.asarray(inputs["weight"], dtype=np.float32)
    tms = [
        np.asarray(inputs[f"t_{n}"], dtype=np.float32)[0]
        * np.asarray(inputs[f"m_{n}"], dtype=np.float32)[0]
        for n in ("bayer", "quad", "nano", "qxq")
    ]
    w5 = np.stack(tms + [w], axis=0)  # (5, CIN, COUT, K, K)
    w5 = w5.reshape(5, NCH, P, COUT, K, K).transpose(2, 0, 1, 4, 5, 3)
    w5 = np.ascontiguousarray(w5)  # (P, 5, NCH, K, K, COUT)
    biasb = np.ascontiguousarray(
        np.asarray(inputs["bias"], dtype=np.float32).reshape(P, 1)
    )
    ab = np.ascontiguousarray(
        np.broadcast_to(
            np.asarray(inputs["prelu_a"], dtype=np.float32).reshape(1, 1), (P, 1)
        )
    )
    in_maps = []
    for i in range(NCORES):
        sl = slice(i * BPC, (i + 1) * BPC)
        in_maps.append(
            {
                "x_sh": xp[sl],
                "w5": w5,
                "featb": np.ascontiguousarray(
                    np.broadcast_to(feat[sl][None], (P, BPC, 4))
                ),
                "biasb": biasb,
                "ab": ab,
            }
        )
    return in_maps


def kernel(**inputs):
    nc = _get_compiled()
    in_maps = _prep_in_maps(inputs)
    res = bass_utils.run_bass_kernel_spmd(nc, in_maps, core_ids=list(range(NCORES)))
    return np.concatenate(
        [res.results[i]["out_sh"] for i in range(NCORES)], axis=0
    )


# revision 10
# speedup vs baseline: 1.0974x; 1.0974x over previous
"""Trainium2 Bass kernel for nn_DeconvBlock (dynamic-weight transposed conv).

Computes, per sample b:
    w_b   = weight + sum_j feature[b,j] * (t_j * m_j)            (weight synthesis)
    out_b = conv_transpose2d(x_b, w_b, stride=2, pad=1, K=4)     (grouped over batch)
    out   = prelu(out_b + bias, a)

Strategy (data-parallel over batch, 8 cores x 2 samples):
  - conv_transpose(stride 2, K=4, P=1) decomposes into 4 output phases
    (py,px) in {0,1}^2; each phase output pixel is a sum of 4 "taps"
    (ky,kx), each tap a 1x1 conv (matmul over CIN=256) of a +-1 shifted x.
  - Weights are synthesized on-device on VectorE with fused
    scalar_tensor_tensor (acc = TM_j * f_bj + acc).
  - Matmuls run as float32r (FP22 multiply, fp32 accumulate) which is
    full-rate on TRN2 for moving dim >= 256.
  - Epilogue: ScalarE adds bias (Identity activation w/ per-partition
    bias), VectorE computes prelu(t) = max(t, a*t) in one fused op while
    interleaving the 4 phases into contiguous output rows for clean DMA.
"""

import numpy as np

import concourse.bass as bass
import concourse.mybir as mybir
from concourse import bacc
from concourse import bass_utils
from concourse.tile import TileContext

B, CIN, COUT, H, W, K, S = 16, 256, 128, 64, 64, 4, 2
NCORES = 8
BPC = B // NCORES  # samples per core
P = 128
NCH = CIN // P     # ic chunks of 128
HP = H + 2         # padded x height/width (zero border of 1)
NROW = 8           # output-phase rows per block
NYB = H // NROW    # row blocks per sample

# phase py -> ((ky, sy), ...): contribution x[y'+sy] * w[ky]
_TAPS = {0: ((1, 0), (3, -1)), 1: ((2, 0), (0, 1))}

_COMPILED = None


def _build():
    f32 = mybir.dt.float32
    f32r = mybir.dt.float32r
    f16 = mybir.dt.float16
    Alu = mybir.AluOpType
    Act = mybir.ActivationFunctionType

    nc = bacc.Bacc(
        "TRN2", target_bir_lowering=False, debug=False, num_devices=NCORES
    )
    x_d = nc.dram_tensor(
        "x_sh", (BPC, NCH, P, HP, HP), f16, kind="ExternalInput"
    ).ap()
    w5_d = nc.dram_tensor("w5", (P, 5, NCH, K, K, COUT), f16, kind="ExternalInput").ap()
    feat_d = nc.dram_tensor("featb", (P, BPC, 4), f32, kind="ExternalInput").ap()
    bias_d = nc.dram_tensor("biasb", (P, 1), f32, kind="ExternalInput").ap()
    a_d = nc.dram_tensor("ab", (P, 1), f32, kind="ExternalInput").ap()
    out_d = nc.dram_tensor(
        "out_sh", (BPC, COUT, H * S, W * S), f32, kind="ExternalOutput"
    ).ap()

    with TileContext(nc) as tc:
        with (
            tc.tile_pool(name="const", bufs=1) as const_pool,
            tc.tile_pool(name="tmj_pool", bufs=3) as tm_pool,
            tc.tile_pool(name="wsyn_pool", bufs=1) as wsyn_pool,
            tc.tile_pool(name="x_pool", bufs=1) as x_pool,
            tc.tile_pool(name="t_pool", bufs=4) as t_pool,
            tc.tile_pool(name="row_pool", bufs=3) as row_pool,
            tc.tile_pool(name="psum", bufs=8, space="PSUM") as psum_pool,
        ):
            feat_t = const_pool.tile([P, BPC, 4], f32)
            nc.sync.dma_start(feat_t[:], feat_d[:])
            bias_t = const_pool.tile([P, 1], f32)
            nc.sync.dma_start(bias_t[:], bias_d[:])
            a_t = const_pool.tile([P, 1], f32)
            nc.sync.dma_start(a_t[:], a_d[:])

            # ---- per-sample weight synthesis on VectorE ----
            # w_syn[s][p, c, ky, kx, oc] = base + sum_j f[s,j] * TM_j
            wsyn = []
            xt = []
            for s in range(BPC):
                w_s = wsyn_pool.tile(
                    [P, NCH, K, K, COUT], f16, name=f"wsyn{s}", tag=f"wsyn{s}"
                )
                wsyn.append(w_s)
                x_s = x_pool.tile(
                    [P, NCH, HP, HP], f16, name=f"xpad{s}", tag=f"xpad{s}"
                )
                xt.append(x_s)
            # chunk 0 weights + sample 0 x first: they gate the first matmuls
            for c in range(NCH):
                for s in range(BPC):
                    nc.sync.dma_start(wsyn[s][:, c], w5_d[:, 4, c])
                for j in range(4):
                    tmj = tm_pool.tile([P, K, K, COUT], f16, name="tmj", tag="tmj")
                    nc.sync.dma_start(tmj[:], w5_d[:, j, c])
                    for s in range(BPC):
                        nc.vector.scalar_tensor_tensor(
                            wsyn[s][:, c],
                            tmj[:],
                            feat_t[:, s, j : j + 1],
                            wsyn[s][:, c],
                            op0=Alu.mult,
                            op1=Alu.add,
                        )
                if c == 0:
                    # x sample 0, banded so the first row-block can start early
                    for cc in range(NCH):
                        nc.sync.dma_start(
                            xt[0][:, cc, 0:33], x_d[0, cc, :, 0:33]
                        )
                    for cc in range(NCH):
                        nc.sync.dma_start(
                            xt[0][:, cc, 33:HP], x_d[0, cc, :, 33:HP]
                        )
            for cc in range(NCH):
                nc.sync.dma_start(xt[1][:, cc], x_d[1, cc])

            # ---- main conv loop ----
            for s in range(BPC):
                for yb in range(NYB):
                    # row_t free layout (y', py, x', px) == out rows
                    # [2*NROW, 2*W] for oy in [2*NROW*yb, 2*NROW*(yb+1))
                    row_t = row_pool.tile(
                        [P, NROW, 2, W, 2], f32, name="row_t", tag="row_t"
                    )
                    for py in (0, 1):
                        for px in (0, 1):
                            ps = psum_pool.tile(
                                [P, NROW, W], f32, name="ps", tag="ps"
                            )
                            k = 0
                            for c in range(NCH):
                                for ky, sy in _TAPS[py]:
                                    for kx, sx in _TAPS[px]:
                                        lhsT = wsyn[s][:, c, ky, kx, :]
                                        y0 = 1 + sy + NROW * yb
                                        x0 = 1 + sx
                                        rhs = xt[s][
                                            :, c, y0 : y0 + NROW, x0 : x0 + W
                                        ]
                                        nc.tensor.matmul(
                                            ps[:],
                                            lhsT,
                                            rhs,
                                            start=(k == 0),
                                            stop=(k == 7),
                                        )
                                        k += 1
                            tt = t_pool.tile([P, NROW, W], f32, name="tt", tag="tt")
                            nc.scalar.activation(
                                tt[:], ps[:], Act.Identity, bias=bias_t[:], scale=1.0
                            )
                            # prelu(t) = max(t, a*t), interleaved into row_t
                            nc.vector.scalar_tensor_tensor(
                                row_t[:, :, py, :, px],
                                tt[:],
                                a_t[:],
                                tt[:],
                                op0=Alu.mult,
                                op1=Alu.max,
                            )
                    oy0 = 2 * NROW * yb
                    nc.sync.dma_start(
                        out_d[s, :, oy0 : oy0 + 2 * NROW, :], row_t[:]
                    )

    nc.compile()
    return nc


def _get_compiled():
    global _COMPILED
    if _COMPILED is None:
        _COMPILED = _build()
    return _COMPILED


def _prep_in_maps(inputs):
    x = np.asarray(inputs["x"], dtype=np.float32)
    xp = np.zeros((B, NCH, P, HP, HP), dtype=np.float16)
    xp[:, :, :, 1 : HP - 1, 1 : HP - 1] = x.reshape(B, NCH, P, H, W)
    feat = np.asarray(inputs["feature"], dtype=np.float32)
    w = np.asarray(inputs["weight"], dtype=np.float32)
    tms = [
        np.asarray(inputs[f"t_{n}"], dtype=np.float32)[0]
        * np.asarray(inputs[f"m_{n}"], dtype=np.float32)[0]
        for n in ("bayer", "quad", "nano", "qxq")
    ]
    w5 = np.stack(tms + [w], axis=0)  # (5, CIN, COUT, K, K)
    w5 = w5.reshape(5, NCH, P, COUT, K, K).transpose(2, 0, 1, 4, 5, 3)
    w5 = np.ascontiguousarray(w5.astype(np.float16))  # (P, 5, NCH, K, K, COUT)
    biasb = np.ascontiguousarray(
        np.asarray(inputs["bias"], dtype=np.float32).reshape(P, 1)
    )
    ab = np.ascontiguousarray(
        np.broadcast_to(
            np.asarray(inputs["prelu_a"], dtype=np.float32).reshape(1, 1), (P, 1)
        )
    )
    in_maps = []
    for i in range(NCORES):
        sl = slice(i * BPC, (i + 1) * BPC)
        in_maps.append(
            {
                "x_sh": xp[sl],
                "w5": w5,
                "featb": np.ascontiguousarray(
                    np.broadcast_to(feat[sl][None], (P, BPC, 4))
                ),
                "biasb": biasb,
                "ab": ab,
            }
        )
    return in_maps


def kernel(**inputs):
    nc = _get_compiled()
    in_maps = _prep_in_maps(inputs)
    res = bass_utils.run_bass_kernel_spmd(nc, in_maps, core_ids=list(range(NCORES)))
    return np.concatenate(
        [res.results[i]["out_sh"] for i in range(NCORES)], axis=0
    )


# revision 11
# speedup vs baseline: 1.1814x; 1.0765x over previous
"""Trainium2 Bass kernel for nn_DeconvBlock (dynamic-weight transposed conv).

Computes, per sample b:
    w_b   = weight + sum_j feature[b,j] * (t_j * m_j)            (weight synthesis)
    out_b = conv_transpose2d(x_b, w_b, stride=2, pad=1, K=4)     (grouped over batch)
    out   = prelu(out_b + bias, a)

Strategy (data-parallel over batch, 8 cores x 2 samples):
  - conv_transpose(stride 2, K=4, P=1) decomposes into 4 output phases
    (py,px) in {0,1}^2; each phase output pixel is a sum of 4 "taps"
    (ky,kx), each tap a 1x1 conv (matmul over CIN=256) of a +-1 shifted x.
  - Weights are synthesized on-device on VectorE with fused
    scalar_tensor_tensor (acc = TM_j * f_bj + acc).
  - Matmuls run as float32r (FP22 multiply, fp32 accumulate) which is
    full-rate on TRN2 for moving dim >= 256.
  - Epilogue: ScalarE adds bias (Identity activation w/ per-partition
    bias), VectorE computes prelu(t) = max(t, a*t) in one fused op while
    interleaving the 4 phases into contiguous output rows for clean DMA.
"""

import numpy as np

import concourse.bass as bass
import concourse.mybir as mybir
from concourse import bacc
from concourse import bass_utils
from concourse.tile import TileContext

B, CIN, COUT, H, W, K, S = 16, 256, 128, 64, 64, 4, 2
NCORES = 8
BPC = B // NCORES  # samples per core
P = 128
NCH = CIN // P     # ic chunks of 128
HP = H + 2         # padded x height/width (zero border of 1)
NROW = 8           # output-phase rows per block
NYB = H // NROW    # row blocks per sample

# phase py -> ((ky, sy), ...): contribution x[y'+sy] * w[ky]
_TAPS = {0: ((1, 0), (3, -1)), 1: ((2, 0), (0, 1))}

_COMPILED = None


def _build():
    f32 = mybir.dt.float32
    f32r = mybir.dt.float32r
    f16 = mybir.dt.float16
    Alu = mybir.AluOpType
    Act = mybir.ActivationFunctionType

    nc = bacc.Bacc(
        "TRN2", target_bir_lowering=False, debug=False, num_devices=NCORES
    )
    x_d = nc.dram_tensor(
        "x_sh", (BPC, NCH, P, HP, HP), f16, kind="ExternalInput"
    ).ap()
    w5_d = nc.dram_tensor("w5", (P, 5, NCH, K, K, COUT), f16, kind="ExternalInput").ap()
    feat_d = nc.dram_tensor("featb", (P, BPC, 4), f16, kind="ExternalInput").ap()
    bias_d = nc.dram_tensor("biasb", (P, 1), f32, kind="ExternalInput").ap()
    a_d = nc.dram_tensor("ab", (P, 1), f32, kind="ExternalInput").ap()
    out_d = nc.dram_tensor(
        "out_sh", (BPC, COUT, H * S, W * S), f32, kind="ExternalOutput"
    ).ap()

    with TileContext(nc) as tc:
        with (
            tc.tile_pool(name="const", bufs=1) as const_pool,
            tc.tile_pool(name="tmj_pool", bufs=1) as tm_pool,
            tc.tile_pool(name="wsyn_pool", bufs=1) as wsyn_pool,
            tc.tile_pool(name="x_pool", bufs=1) as x_pool,
            tc.tile_pool(name="t_pool", bufs=4) as t_pool,
            tc.tile_pool(name="row_pool", bufs=3) as row_pool,
            tc.tile_pool(name="psum", bufs=8, space="PSUM") as psum_pool,
        ):
            feat_t = const_pool.tile([P, BPC, 4], f16)
            nc.sync.dma_start(feat_t[:], feat_d[:])
            bias_t = const_pool.tile([P, 1], f32)
            nc.sync.dma_start(bias_t[:], bias_d[:])
            a_t = const_pool.tile([P, 1], f32)
            nc.sync.dma_start(a_t[:], a_d[:])

            # ---- per-sample weight synthesis on VectorE ----
            # w_syn[s][p, c, ky, kx, oc] = base + sum_j f[s,j] * TM_j
            wsyn = []
            xt = []
            for s in range(BPC):
                w_s = wsyn_pool.tile(
                    [P, NCH, K, K, COUT], f16, name=f"wsyn{s}", tag=f"wsyn{s}"
                )
                wsyn.append(w_s)
                x_s = x_pool.tile(
                    [P, NCH, HP, HP], f16, name=f"xpad{s}", tag=f"xpad{s}"
                )
                xt.append(x_s)
            # DMA order = priority order: sample-0 chunk-0 weights gate the
            # first matmuls, then x bands of sample 0, then the rest.
            tmt = {}
            for c in range(NCH):
                nc.sync.dma_start(wsyn[0][:, c], w5_d[:, 4, c])
                for j in range(4):
                    tmj = tm_pool.tile(
                        [P, K, K, COUT], f16, name=f"tm{c}{j}", tag=f"tm{c}{j}"
                    )
                    tmt[(c, j)] = tmj
                    nc.sync.dma_start(tmj[:], w5_d[:, j, c])
                if c == 0:
                    # x sample 0, banded so the first row-block can start early
                    for cc in range(NCH):
                        nc.sync.dma_start(
                            xt[0][:, cc, 0:33], x_d[0, cc, :, 0:33]
                        )
            # synthesis: sample 0 fully first (it alone gates the first MMs)
            for s in range(BPC):
                if s == 1:
                    for c in range(NCH):
                        nc.sync.dma_start(wsyn[1][:, c], w5_d[:, 4, c])
                for c in range(NCH):
                    for j in range(4):
                        nc.vector.scalar_tensor_tensor(
                            wsyn[s][:, c],
                            tmt[(c, j)][:],
                            feat_t[:, s, j : j + 1],
                            wsyn[s][:, c],
                            op0=Alu.mult,
                            op1=Alu.add,
                        )
                if s == 0:
                    for cc in range(NCH):
                        nc.sync.dma_start(
                            xt[0][:, cc, 33:HP], x_d[0, cc, :, 33:HP]
                        )
                    for cc in range(NCH):
                        nc.sync.dma_start(xt[1][:, cc], x_d[1, cc])

            # ---- main conv loop ----
            for s in range(BPC):
                for yb in range(NYB):
                    # row_t free layout (y', py, x', px) == out rows
                    # [2*NROW, 2*W] for oy in [2*NROW*yb, 2*NROW*(yb+1))
                    row_t = row_pool.tile(
                        [P, NROW, 2, W, 2], f32, name="row_t", tag="row_t"
                    )
                    for py in (0, 1):
                        for px in (0, 1):
                            ps = psum_pool.tile(
                                [P, NROW, W], f32, name="ps", tag="ps"
                            )
                            k = 0
                            for c in range(NCH):
                                for ky, sy in _TAPS[py]:
                                    for kx, sx in _TAPS[px]:
                                        lhsT = wsyn[s][:, c, ky, kx, :]
                                        y0 = 1 + sy + NROW * yb
                                        x0 = 1 + sx
                                        rhs = xt[s][
                                            :, c, y0 : y0 + NROW, x0 : x0 + W
                                        ]
                                        nc.tensor.matmul(
                                            ps[:],
                                            lhsT,
                                            rhs,
                                            start=(k == 0),
                                            stop=(k == 7),
                                        )
                                        k += 1
                            tt = t_pool.tile([P, NROW, W], f32, name="tt", tag="tt")
                            nc.scalar.activation(
                                tt[:], ps[:], Act.Identity, bias=bias_t[:], scale=1.0
                            )
                            # prelu(t) = max(t, a*t), interleaved into row_t
                            nc.vector.scalar_tensor_tensor(
                                row_t[:, :, py, :, px],
                                tt[:],
                                a_t[:],
                                tt[:],
                                op0=Alu.mult,
                                op1=Alu.max,
                            )
                    oy0 = 2 * NROW * yb
                    nc.sync.dma_start(
                        out_d[s, :, oy0 : oy0 + 2 * NROW, :], row_t[:]
                    )

    nc.compile()
    return nc


def _get_compiled():
    global _COMPILED
    if _COMPILED is None:
        _COMPILED = _build()
    return _COMPILED


def _prep_in_maps(inputs):
    x = np.asarray(inputs["x"], dtype=np.float32)
    xp = np.zeros((B, NCH, P, HP, HP), dtype=np.float16)
    xp[:, :, :, 1 : HP - 1, 1 : HP - 1] = x.reshape(B, NCH, P, H, W)
    feat = np.asarray(inputs["feature"], dtype=np.float32)
    w = np.asarray(inputs["weight"], dtype=np.float32)
    tms = [
        np.asarray(inputs[f"t_{n}"], dtype=np.float32)[0]
        * np.asarray(inputs[f"m_{n}"], dtype=np.float32)[0]
        for n in ("bayer", "quad", "nano", "qxq")
    ]
    w5 = np.stack(tms + [w], axis=0)  # (5, CIN, COUT, K, K)
    w5 = w5.reshape(5, NCH, P, COUT, K, K).transpose(2, 0, 1, 4, 5, 3)
    w5 = np.ascontiguousarray(w5.astype(np.float16))  # (P, 5, NCH, K, K, COUT)
    biasb = np.ascontiguousarray(
        np.asarray(inputs["bias"], dtype=np.float32).reshape(P, 1)
    )
    ab = np.ascontiguousarray(
        np.broadcast_to(
            np.asarray(inputs["prelu_a"], dtype=np.float32).reshape(1, 1), (P, 1)
        )
    )
    in_maps = []
    for i in range(NCORES):
        sl = slice(i * BPC, (i + 1) * BPC)
        in_maps.append(
            {
                "x_sh": xp[sl],
                "w5": w5,
                "featb": np.ascontiguousarray(
                    np.broadcast_to(feat[sl][None], (P, BPC, 4))
                ).astype(np.float16),
                "biasb": biasb,
                "ab": ab,
            }
        )
    return in_maps


def kernel(**inputs):
    nc = _get_compiled()
    in_maps = _prep_in_maps(inputs)
    res = bass_utils.run_bass_kernel_spmd(nc, in_maps, core_ids=list(range(NCORES)))
    return np.concatenate(
        [res.results[i]["out_sh"] for i in range(NCORES)], axis=0
    )


# revision 13
# speedup vs baseline: 1.2452x; 1.0541x over previous
"""Trainium2 Bass kernel for nn_DeconvBlock (dynamic-weight transposed conv).

Computes, per sample b:
    w_b   = weight + sum_j feature[b,j] * (t_j * m_j)            (weight synthesis)
    out_b = conv_transpose2d(x_b, w_b, stride=2, pad=1, K=4)     (grouped over batch)
    out   = prelu(out_b + bias, a)

Strategy (data-parallel over batch, 8 cores x 2 samples):
  - conv_transpose(stride 2, K=4, P=1) decomposes into 4 output phases
    (py,px) in {0,1}^2; each phase output pixel is a sum of 4 "taps"
    (ky,kx), each tap a 1x1 conv (matmul over CIN=256) of a +-1 shifted x.
  - Weights are synthesized on-device on VectorE with fused
    scalar_tensor_tensor (acc = TM_j * f_bj + acc).
  - Matmuls run as float32r (FP22 multiply, fp32 accumulate) which is
    full-rate on TRN2 for moving dim >= 256.
  - Epilogue: ScalarE adds bias (Identity activation w/ per-partition
    bias), VectorE computes prelu(t) = max(t, a*t) in one fused op while
    interleaving the 4 phases into contiguous output rows for clean DMA.
"""

import numpy as np

import concourse.bass as bass
import concourse.mybir as mybir
from concourse import bacc
from concourse import bass_utils
from concourse.tile import TileContext

B, CIN, COUT, H, W, K, S = 16, 256, 128, 64, 64, 4, 2
NCORES = 8
BPC = B // NCORES  # samples per core
P = 128
NCH = CIN // P     # ic chunks of 128
HP = H + 2         # padded x height/width (zero border of 1)
NROW = 8           # output-phase rows per block
NYB = H // NROW    # row blocks per sample

# phase py -> ((ky, sy), ...): contribution x[y'+sy] * w[ky]
_TAPS = {0: ((1, 0), (3, -1)), 1: ((2, 0), (0, 1))}

_COMPILED = None


def _build():
    f32 = mybir.dt.float32
    f32r = mybir.dt.float32r
    f16 = mybir.dt.float16
    Alu = mybir.AluOpType
    Act = mybir.ActivationFunctionType

    nc = bacc.Bacc(
        "TRN2", target_bir_lowering=False, debug=False, num_devices=NCORES
    )
    x_d = nc.dram_tensor(
        "x_sh", (BPC, NCH, P, HP, HP), f16, kind="ExternalInput"
    ).ap()
    w5_d = nc.dram_tensor("w5", (P, 5, NCH, K, K, COUT), f16, kind="ExternalInput").ap()
    feat_d = nc.dram_tensor("featb", (P, BPC, 4), f32, kind="ExternalInput").ap()
    bias_d = nc.dram_tensor("biasb", (P, 1), f32, kind="ExternalInput").ap()
    a_d = nc.dram_tensor("ab", (P, 1), f32, kind="ExternalInput").ap()
    out_d = nc.dram_tensor(
        "out_sh", (BPC, COUT, H * S, W * S), f32, kind="ExternalOutput"
    ).ap()

    with TileContext(nc) as tc:
        with (
            tc.tile_pool(name="const", bufs=1) as const_pool,
            tc.tile_pool(name="tmj_pool", bufs=1) as tm_pool,
            tc.tile_pool(name="wsyn_pool", bufs=1) as wsyn_pool,
            tc.tile_pool(name="x_pool", bufs=1) as x_pool,
            tc.tile_pool(name="t_pool", bufs=6) as t_pool,
            tc.tile_pool(name="sm_pool", bufs=3) as sm_pool,
            tc.tile_pool(name="row_pool", bufs=4) as row_pool,
            tc.tile_pool(name="psum", bufs=8, space="PSUM") as psum_pool,
        ):
            feat_t = const_pool.tile([P, BPC, 4], f32)
            nc.sync.dma_start(feat_t[:], feat_d[:])
            bias_t = const_pool.tile([P, 1], f32)
            a_t = const_pool.tile([P, 1], f32)

            # ---- per-sample weight synthesis on VectorE ----
            # w_syn[s][p, c, ky, kx, oc] = base + sum_j f[s,j] * TM_j
            wsyn = []
            xt = []
            for s in range(BPC):
                w_s = wsyn_pool.tile(
                    [P, NCH, K, K, COUT], f16, name=f"wsyn{s}", tag=f"wsyn{s}"
                )
                wsyn.append(w_s)
                x_s = x_pool.tile(
                    [P, NCH, HP, HP], f16, name=f"xpad{s}", tag=f"xpad{s}"
                )
                xt.append(x_s)
            # DMA order = priority order: sample-0 chunk-0 weights gate the
            # first matmuls, then x bands of sample 0, then the rest.
            tmt = {}
            for c in range(NCH):
                nc.sync.dma_start(wsyn[0][:, c], w5_d[:, 4, c])
                for j in range(4):
                    tmj = tm_pool.tile(
                        [P, K, K, COUT], f16, name=f"tm{c}{j}", tag=f"tm{c}{j}"
                    )
                    tmt[(c, j)] = tmj
                    nc.sync.dma_start(tmj[:], w5_d[:, j, c])
                if c == 0:
                    # x sample 0 rows 0:33: gates the first row-blocks
                    for cc in range(NCH):
                        nc.sync.dma_start(
                            xt[0][:, cc, 0:33], x_d[0, cc, :, 0:33]
                        )
            nc.sync.dma_start(bias_t[:], bias_d[:])
            nc.sync.dma_start(a_t[:], a_d[:])

            # synthesis: sample 0 fully first (it alone gates the first MMs).
            # j=0 fused on VectorE; j=1..3 scaled on ScalarE (sm = TM_j*f) and
            # accumulated on VectorE with 2x-mode fp16 tensor_tensor adds.
            def synth(s, c):
                nc.vector.scalar_tensor_tensor(
                    wsyn[s][:, c],
                    tmt[(c, 0)][:],
                    feat_t[:, s, 0:1],
                    wsyn[s][:, c],
                    op0=Alu.mult,
                    op1=Alu.add,
                )
                for j in range(1, 4):
                    sm = sm_pool.tile([P, K, K, COUT], f16, name="sm", tag="sm")
                    nc.scalar.activation(
                        sm[:],
                        tmt[(c, j)][:],
                        Act.Identity,
                        scale=feat_t[:, s, j : j + 1],
                    )
                    nc.vector.tensor_tensor(
                        wsyn[s][:, c], wsyn[s][:, c], sm[:], op=Alu.add
                    )

            for c in range(NCH):
                synth(0, c)
                if c == 0:
                    for cc in range(NCH):
                        nc.sync.dma_start(
                            xt[0][:, cc, 33:HP], x_d[0, cc, :, 33:HP]
                        )
            for cc in range(NCH):
                nc.sync.dma_start(xt[1][:, cc], x_d[1, cc])
            for c in range(NCH):
                nc.sync.dma_start(wsyn[1][:, c], w5_d[:, 4, c])
            for c in range(NCH):
                synth(1, c)

            # ---- main conv loop ----
            for s in range(BPC):
                for yb in range(NYB):
                    # row_t free layout (y', py, x', px) == out rows
                    # [2*NROW, 2*W] for oy in [2*NROW*yb, 2*NROW*(yb+1))
                    row_t = row_pool.tile(
                        [P, NROW, 2, W, 2], f32, name="row_t", tag="row_t"
                    )
                    for py in (0, 1):
                        for px in (0, 1):
                            ps = psum_pool.tile(
                                [P, NROW, W], f32, name="ps", tag="ps"
                            )
                            k = 0
                            for c in range(NCH):
                                for ky, sy in _TAPS[py]:
                                    for kx, sx in _TAPS[px]:
                                        lhsT = wsyn[s][:, c, ky, kx, :]
                                        y0 = 1 + sy + NROW * yb
                                        x0 = 1 + sx
                                        rhs = xt[s][
                                            :, c, y0 : y0 + NROW, x0 : x0 + W
                                        ]
                                        nc.tensor.matmul(
                                            ps[:],
                                            lhsT,
                                            rhs,
                                            start=(k == 0),
                                            stop=(k == 7),
                                        )
                                        k += 1
                            tt = t_pool.tile([P, NROW, W], f32, name="tt", tag="tt")
                            nc.scalar.activation(
                                tt[:], ps[:], Act.Identity, bias=bias_t[:], scale=1.0
                            )
                            # prelu(t) = max(t, a*t), interleaved into row_t
                            nc.vector.scalar_tensor_tensor(
                                row_t[:, :, py, :, px],
                                tt[:],
                                a_t[:],
                                tt[:],
                                op0=Alu.mult,
                                op1=Alu.max,
                            )
                        if px == 1:
                            oy0 = 2 * NROW * yb + py
                            nc.sync.dma_start(
                                out_d[s, :, oy0 : oy0 + 2 * NROW - 1 : 2, :],
                                row_t[:, :, py],
                            )


    nc.compile()
    return nc


def _get_compiled():
    global _COMPILED
    if _COMPILED is None:
        _COMPILED = _build()
    return _COMPILED


def _prep_in_maps(inputs):
    x = np.asarray(inputs["x"], dtype=np.float32)
    xp = np.zeros((B, NCH, P, HP, HP), dtype=np.float16)
    xp[:, :, :, 1 : HP - 1, 1 : HP - 1] = x.reshape(B, NCH, P, H, W)
    feat = np.asarray(inputs["feature"], dtype=np.float32)
    w = np.asarray(inputs["weight"], dtype=np.float32)
    tms = [
        np.asarray(inputs[f"t_{n}"], dtype=np.float32)[0]
        * np.asarray(inputs[f"m_{n}"], dtype=np.float32)[0]
        for n in ("bayer", "quad", "nano", "qxq")
    ]
    w5 = np.stack(tms + [w], axis=0)  # (5, CIN, COUT, K, K)
    w5 = w5.reshape(5, NCH, P, COUT, K, K).transpose(2, 0, 1, 4, 5, 3)
    w5 = np.ascontiguousarray(w5.astype(np.float16))  # (P, 5, NCH, K, K, COUT)
    biasb = np.ascontiguousarray(
        np.asarray(inputs["bias"], dtype=np.float32).reshape(P, 1)
    )
    ab = np.ascontiguousarray(
        np.broadcast_to(
            np.asarray(inputs["prelu_a"], dtype=np.float32).reshape(1, 1), (P, 1)
        )
    )
    in_maps = []
    for i in range(NCORES):
        sl = slice(i * BPC, (i + 1) * BPC)
        in_maps.append(
            {
                "x_sh": xp[sl],
                "w5": w5,
                "featb": np.ascontiguousarray(
                    np.broadcast_to(feat[sl][None], (P, BPC, 4))
                ),
                "biasb": biasb,
                "ab": ab,
            }
        )
    return in_maps


def kernel(**inputs):
    nc = _get_compiled()
    in_maps = _prep_in_maps(inputs)
    res = bass_utils.run_bass_kernel_spmd(nc, in_maps, core_ids=list(range(NCORES)))
    return np.concatenate(
        [res.results[i]["out_sh"] for i in range(NCORES)], axis=0
    )


# revision 19
# speedup vs baseline: 1.2677x; 1.0180x over previous
"""Trainium2 Bass kernel for nn_DeconvBlock (dynamic-weight transposed conv).

Computes, per sample b:
    w_b   = weight + sum_j feature[b,j] * (t_j * m_j)            (weight synthesis)
    out_b = conv_transpose2d(x_b, w_b, stride=2, pad=1, K=4)     (grouped over batch)
    out   = prelu(out_b + bias, a)

Strategy (data-parallel over batch, 8 cores x 2 samples):
  - conv_transpose(stride 2, K=4, P=1) decomposes into 4 output phases
    (py,px) in {0,1}^2; each phase output pixel is a sum of 4 "taps"
    (ky,kx), each tap a 1x1 conv (matmul over CIN=256) of a +-1 shifted x.
  - Weights are synthesized on-device on VectorE with fused
    scalar_tensor_tensor (acc = TM_j * f_bj + acc).
  - Matmuls run as float32r (FP22 multiply, fp32 accumulate) which is
    full-rate on TRN2 for moving dim >= 256.
  - Epilogue: ScalarE adds bias (Identity activation w/ per-partition
    bias), VectorE computes prelu(t) = max(t, a*t) in one fused op while
    interleaving the 4 phases into contiguous output rows for clean DMA.
"""

import numpy as np

import concourse.bass as bass
import concourse.mybir as mybir
from concourse import bacc
from concourse import bass_utils
from concourse.tile import TileContext

B, CIN, COUT, H, W, K, S = 16, 256, 128, 64, 64, 4, 2
NCORES = 8
BPC = B // NCORES  # samples per core
P = 128
NCH = CIN // P     # ic chunks of 128
HP = H + 2         # padded x height/width (zero border of 1)
NROW = 8           # output-phase rows per block
NYB = H // NROW    # row blocks per sample

# phase py -> ((ky, sy), ...): contribution x[y'+sy] * w[ky]
_TAPS = {0: ((1, 0), (3, -1)), 1: ((2, 0), (0, 1))}

_COMPILED = None


def _build():
    f32 = mybir.dt.float32
    f32r = mybir.dt.float32r
    f16 = mybir.dt.float16
    Alu = mybir.AluOpType
    Act = mybir.ActivationFunctionType

    nc = bacc.Bacc(
        "TRN2", target_bir_lowering=False, debug=False, num_devices=NCORES
    )
    x_d = nc.dram_tensor(
        "x_sh", (BPC, NCH, P, HP, HP), f16, kind="ExternalInput"
    ).ap()
    w5_d = nc.dram_tensor("w5", (P, 5, NCH, K, K, COUT), f16, kind="ExternalInput").ap()
    feat_d = nc.dram_tensor("featb", (P, BPC, 4), f32, kind="ExternalInput").ap()
    bias_d = nc.dram_tensor("biasb", (P, 1), f32, kind="ExternalInput").ap()
    a_d = nc.dram_tensor("ab", (P, 1), f32, kind="ExternalInput").ap()
    out_d = nc.dram_tensor(
        "out_sh", (BPC, COUT, H * S, W * S), f32, kind="ExternalOutput"
    ).ap()

    with TileContext(nc) as tc:
        with (
            tc.tile_pool(name="const", bufs=1) as const_pool,
            tc.tile_pool(name="tmj_pool", bufs=1) as tm_pool,
            tc.tile_pool(name="wsyn_pool", bufs=1) as wsyn_pool,
            tc.tile_pool(name="x_pool", bufs=1) as x_pool,
            tc.tile_pool(name="t_pool", bufs=6) as t_pool,
            tc.tile_pool(name="sm_pool", bufs=3) as sm_pool,
            tc.tile_pool(name="row_pool", bufs=4) as row_pool,
            tc.tile_pool(name="psum", bufs=8, space="PSUM") as psum_pool,
        ):
            feat_t = const_pool.tile([P, BPC, 4], f32)
            nc.sync.dma_start(feat_t[:], feat_d[:])
            bias_t = const_pool.tile([P, 1], f32)
            a_t = const_pool.tile([P, 1], f32)
            # warm the ScalarE activation table (Identity) during startup DMAs
            scratch_t = const_pool.tile([P, 1], f32)
            nc.vector.memset(scratch_t[:], 0.0)
            nc.scalar.activation(scratch_t[:], scratch_t[:], Act.Identity, scale=1.0)

            # ---- per-sample weight synthesis on VectorE ----
            # w_syn[s][p, c, ky, kx, oc] = base + sum_j f[s,j] * TM_j
            wsyn = []
            xt = []
            for s in range(BPC):
                w_s = wsyn_pool.tile(
                    [P, NCH, K, K, COUT], f16, name=f"wsyn{s}", tag=f"wsyn{s}"
                )
                wsyn.append(w_s)
                x_s = x_pool.tile(
                    [P, NCH, HP, HP], f16, name=f"xpad{s}", tag=f"xpad{s}"
                )
                xt.append(x_s)
            # Startup DMAs scattered across engine queues so the transfers run
            # in parallel (each engine's dynamic HW queue is FIFO-serial).
            # Critical chain: tm(c0,j0) -> stt -> tm(c0,j1..3) scalings.
            tmt = {}
            for c in range(NCH):
                for j in range(4):
                    tmt[(c, j)] = tm_pool.tile(
                        [P, K, K, COUT], f16, name=f"tm{c}{j}", tag=f"tm{c}{j}"
                    )
            nc.sync.dma_start(wsyn[0][:, 0], w5_d[:, 4, 0])
            for j in range(4):
                nc.sync.dma_start(tmt[(0, j)][:], w5_d[:, j, 0])
            # x sample 0 rows 0:33 gate the first row-blocks
            nc.sync.dma_start(xt[0][:, 0, 0:33], x_d[0, 0, :, 0:33])
            nc.sync.dma_start(xt[0][:, 1, 0:33], x_d[0, 1, :, 0:33])
            # chunk-1 weights for sample 0
            nc.sync.dma_start(wsyn[0][:, 1], w5_d[:, 4, 1])
            for j in range(4):
                nc.sync.dma_start(tmt[(1, j)][:], w5_d[:, j, 1])
            nc.sync.dma_start(bias_t[:], bias_d[:])
            nc.sync.dma_start(a_t[:], a_d[:])

            # synthesis: sample 0 fully first (it alone gates the first MMs).
            # j=0 fused on VectorE; j=1..3 scaled on ScalarE (sm = TM_j*f) and
            # accumulated on VectorE with 2x-mode fp16 tensor_tensor adds.
            def synth(s, c):
                nc.vector.scalar_tensor_tensor(
                    wsyn[s][:, c],
                    tmt[(c, 0)][:],
                    feat_t[:, s, 0:1],
                    wsyn[s][:, c],
                    op0=Alu.mult,
                    op1=Alu.add,
                )
                for j in range(1, 4):
                    sm = sm_pool.tile([P, K, K, COUT], f16, name="sm", tag="sm")
                    nc.scalar.activation(
                        sm[:],
                        tmt[(c, j)][:],
                        Act.Identity,
                        scale=feat_t[:, s, j : j + 1],
                    )
                    nc.vector.tensor_tensor(
                        wsyn[s][:, c], wsyn[s][:, c], sm[:], op=Alu.add
                    )

            for c in range(NCH):
                synth(0, c)
                if c == 0:
                    nc.sync.dma_start(xt[0][:, 0, 33:HP], x_d[0, 0, :, 33:HP])
                    nc.sync.dma_start(xt[0][:, 1, 33:HP], x_d[0, 1, :, 33:HP])
            nc.sync.dma_start(xt[1][:, 0], x_d[1, 0])
            nc.sync.dma_start(xt[1][:, 1], x_d[1, 1])
            nc.sync.dma_start(wsyn[1][:, 0], w5_d[:, 4, 0])
            nc.sync.dma_start(wsyn[1][:, 1], w5_d[:, 4, 1])
            for c in range(NCH):
                synth(1, c)

            # ---- main conv loop ----
            for s in range(BPC):
                for yb in range(NYB):
                    # row_t free layout (y', py, x', px) == out rows
                    # [2*NROW, 2*W] for oy in [2*NROW*yb, 2*NROW*(yb+1))
                    row_t = row_pool.tile(
                        [P, NROW, 2, W, 2], f32, name="row_t", tag="row_t"
                    )
                    for py in (0, 1):
                        for px in (0, 1):
                            ps = psum_pool.tile(
                                [P, NROW, W], f32, name="ps", tag="ps"
                            )
                            k = 0
                            for c in range(NCH):
                                for ky, sy in _TAPS[py]:
                                    for kx, sx in _TAPS[px]:
                                        lhsT = wsyn[s][:, c, ky, kx, :]
                                        y0 = 1 + sy + NROW * yb
                                        x0 = 1 + sx
                                        rhs = xt[s][
                                            :, c, y0 : y0 + NROW, x0 : x0 + W
                                        ]
                                        nc.tensor.matmul(
                                            ps[:],
                                            lhsT,
                                            rhs,
                                            start=(k == 0),
                                            stop=(k == 7),
                                        )
                                        k += 1
                            tt = t_pool.tile([P, NROW, W], f32, name="tt", tag="tt")
                            nc.scalar.activation(
                                tt[:], ps[:], Act.Identity, bias=bias_t[:], scale=1.0
                            )
                            # prelu(t) = max(t, a*t), interleaved into row_t
                            nc.vector.scalar_tensor_tensor(
                                row_t[:, :, py, :, px],
                                tt[:],
                                a_t[:],
                                tt[:],
                                op0=Alu.mult,
                                op1=Alu.max,
                            )
                        if px == 1:
                            oy0 = 2 * NROW * yb + py
                            nc.sync.dma_start(
                                out_d[s, :, oy0 : oy0 + 2 * NROW - 1 : 2, :],
                                row_t[:, :, py],
                            )


    nc.compile()
    return nc


def _get_compiled():
    global _COMPILED
    if _COMPILED is None:
        _COMPILED = _build()
    return _COMPILED


def _prep_in_maps(inputs):
    x = np.asarray(inputs["x"], dtype=np.float32)
    xp = np.zeros((B, NCH, P, HP, HP), dtype=np.float16)
    xp[:, :, :, 1 : HP - 1, 1 : HP - 1] = x.reshape(B, NCH, P, H, W)
    feat = np.asarray(inputs["feature"], dtype=np.float32)
    w = np.asarray(inputs["weight"], dtype=np.float32)
    tms = [
        np.asarray(inputs[f"t_{n}"], dtype=np.float32)[0]
        * np.asarray(inputs[f"m_{n}"], dtype=np.float32)[0]
        for n in ("bayer", "quad", "nano", "qxq")
    ]
    w5 = np.stack(tms + [w], axis=0)  # (5, CIN, COUT, K, K)
    w5 = w5.reshape(5, NCH, P, COUT, K, K).transpose(2, 0, 1, 4, 5, 3)
    w5 = np.ascontiguousarray(w5.astype(np.float16))  # (P, 5, NCH, K, K, COUT)
    biasb = np.ascontiguousarray(
        np.asarray(inputs["bias"], dtype=np.float32).reshape(P, 1)
    )
    ab = np.ascontiguousarray(
        np.broadcast_to(
            np.asarray(inputs["prelu_a"], dtype=np.float32).reshape(1, 1), (P, 1)
        )
    )
    in_maps = []
    for i in range(NCORES):
        sl = slice(i * BPC, (i + 1) * BPC)
        in_maps.append(
            {
                "x_sh": xp[sl],
                "w5": w5,
                "featb": np.ascontiguousarray(
                    np.broadcast_to(feat[sl][None], (P, BPC, 4))
                ),
                "biasb": biasb,
                "ab": ab,
            }
        )
    return in_maps


def kernel(**inputs):
    nc = _get_compiled()
    in_maps = _prep_in_maps(inputs)
    res = bass_utils.run_bass_kernel_spmd(nc, in_maps, core_ids=list(range(NCORES)))
    return np.concatenate(
        [res.results[i]["out_sh"] for i in range(NCORES)], axis=0
    )
